# revision 2
# baseline (speedup 1.0000x reference)
"""Causal self-attention Trainium2 kernel (8-core head-parallel), v2.

Sharding identical to v1 (2 heads x 2 batches per core, QKV column-parallel,
c_proj row-parallel with host-side reduce). Differences vs v1:
  - QK^T for the two heads runs CONCURRENTLY on the PE as two 64-row tiles
    (tile_position (0,0)/(64,0)) writing one [128, 1024] S_pair psum tile
    (h0 cols 0:512, h1 cols 512:1024).
  - Attention q-windows are 512 wide; one exp instruction covers both heads.
  - Softmax normalization: 1/Z on DVE (reciprocal_approx_fast) + a single
    K=2 block-ones broadcast matmul per window; no LN/EXP scalar chain.
  - c_proj partials are DMA'd directly from PSUM to DRAM in fp32.
  - xt DMA is chunk-major (b0 first) so attention starts early.
"""

import math

import numpy as np
import ml_dtypes

import concourse.bass as bass
from concourse import bacc
import concourse.mybir as mybir
from concourse.tile import TileContext
from concourse.bass_utils import run_bass_kernel_spmd

BF16 = mybir.dt.bfloat16
F32 = mybir.dt.float32
NPBF16 = ml_dtypes.bfloat16

P = 128
B, T, C = 2, 2048, 1024
H, D = 16, 64
NCORES = 8
HPC = H // NCORES          # heads per core
TOK = B * T                # 4096 flattened tokens (b-major)
NCT = C // P               # 8 contraction tiles for the projections
QW = 512                   # q window width
NW = T // QW               # 4 windows per batch elem
NST = T // P               # 16 s-tiles per batch elem
EXP_BIAS = -4.0            # exp(s - 4): cancels in normalization, guards tail


def _patch_act_tables():
    """Force every exp/ln activation onto the single table set that contains
    both, so the kernel never pays mid-stream ACT_TABLE_LOAD switches."""
    import concourse.bacc as bacc_mod
    if getattr(bacc_mod, "_act_tables_patched", False):
        return
    orig = bacc_mod.get_activation_tables
    EXP = mybir.ActivationFunctionType.Exp
    LN = mybir.ActivationFunctionType.Ln

    def patched(arch):
        t = orig(arch)
        if any(EXP in f and LN in f for f in t.values()):
            for name, fns in t.items():
                if "natural_log_exp" not in name and (EXP in fns or LN in fns):
                    t[name] = fns - {EXP, LN}
        return t

    bacc_mod.get_activation_tables = patched
    bacc_mod._act_tables_patched = True


def build_nc(with_bias: bool) -> bacc.Bacc:
    _patch_act_tables()
    nc = bacc.Bacc(None, target_bir_lowering=False)

    xt = nc.dram_tensor("xt", [C, TOK], BF16, kind="ExternalInput")
    wqkv = nc.dram_tensor("wqkv", [C, 3 * P], BF16, kind="ExternalInput")
    wp = nc.dram_tensor("wp", [P, C], BF16, kind="ExternalInput")
    tri = nc.dram_tensor("tri", [P, P], BF16, kind="ExternalInput")
    ident = nc.dram_tensor("ident", [P, P], BF16, kind="ExternalInput")

    if with_bias:
        bqkv = nc.dram_tensor("bqkv", [1, 3 * P], BF16, kind="ExternalInput")
        ones512 = nc.dram_tensor("ones512", [1, 512], BF16, kind="ExternalInput")
    outT = nc.dram_tensor("outT", [B, C, T], BF16, kind="ExternalOutput")

    EXP = mybir.ActivationFunctionType.Exp

    with TileContext(nc) as tc:
        with (
            tc.tile_pool(name="consts", bufs=1) as consts,
            tc.tile_pool(name="px", bufs=1) as px,
            tc.tile_pool(name="pqkv", bufs=1) as pqkv,
            tc.tile_pool(name="py", bufs=1) as py,
            tc.tile_pool(name="pwork", bufs=2) as pwork,
            tc.tile_pool(name="ps_s", bufs=2, space="PSUM") as ps_s,
            tc.tile_pool(name="ps_y", bufs=2, space="PSUM") as ps_y,
            tc.tile_pool(name="ps_c", bufs=2, space="PSUM") as ps_c,
        ):
            # ---- input loads: xt chunk 0 first (it gates the first QKV),
            # then weights/consts, then the remaining chunks ----
            xt_sb = px.tile([P, NCT, TOK], BF16)

            def xt_dma(b, ck):
                lo = b * T + ck * 512
                for ct in range(NCT):
                    nc.sync.dma_start(
                        xt_sb[:, ct, lo:lo + 512],
                        xt[ct * P:(ct + 1) * P, lo:lo + 512])

            xt_dma(0, 0)
            wqkv_sb = consts.tile([P, NCT, 3 * P], BF16)
            for ct in range(NCT):
                nc.sync.dma_start(wqkv_sb[:, ct, :], wqkv[ct * P:(ct + 1) * P, :])
            wp_sb = consts.tile([P, C], BF16)
            nc.sync.dma_start(wp_sb, wp[:, :])
            tri_sb = consts.tile([P, P], BF16)
            nc.sync.dma_start(tri_sb, tri[:, :])
            ident_sb = consts.tile([P, P], BF16)
            nc.sync.dma_start(ident_sb, ident[:, :])
            expb = consts.tile([P, 1], F32)
            nc.vector.memset(expb, EXP_BIAS)
            if with_bias:
                bqkv_sb = consts.tile([1, 3 * P], BF16)
                nc.sync.dma_start(bqkv_sb, bqkv[:, :])
                ones512_sb = consts.tile([1, 512], BF16)
                nc.sync.dma_start(ones512_sb, ones512[:, :])
            for ck in range(1, 4):
                xt_dma(0, ck)
            for ck in range(4):
                xt_dma(1, ck)

            # qk slabs: ft=0 -> q^T (rows: h0 0:64, h1 64:128), ft=1 -> k^T
            qkT = pqkv.tile([P, 2, TOK], BF16)
            vT = pqkv.tile([P, TOK], BF16)
            # V tiles [s, d] with 64 ones columns (64:128) so the AV matmul
            # replicates the softmax denominator Z across partitions 64:127;
            # index = (b*HPC+h)*NST + st
            V = pqkv.tile([P, B * HPC * NST, P], BF16)
            nc.vector.memset(V, 1.0)
            yT = py.tile([P, B, T], BF16)

            # ---- PE unit definitions ----
            def dummy_mms(n):
                for _ in range(n):
                    scratch = ps_c.tile([P, 512], F32, tag="c",
                                        name="warm")
                    nc.tensor.matmul(
                        scratch[:, 0:384],
                        wqkv_sb[:, 0, 0:P],
                        wqkv_sb[:, 0, :],
                        start=True, stop=True,
                    )

            def dummy_heartbeat():
                # keep the PE HAM counter busy without touching PSUM
                nc.tensor.ldweights(tri_sb[:, :])

            def qkv_unit(b, ft, ck):
                """One [128, 512] slab of q^T/k^T/v^T for batch b, chunk ck."""
                tcc = b * 4 + ck
                pt = ps_c.tile([P, 512], F32, tag="c", name=f"qkvps_{b}_{ft}_{ck}")
                for ct in range(NCT):
                    nc.tensor.matmul(
                        pt,
                        wqkv_sb[:, ct, ft * P:(ft + 1) * P],
                        xt_sb[:, ct, tcc * 512:(tcc + 1) * 512],
                        start=(ct == 0),
                        stop=(ct == NCT - 1 and not with_bias),
                    )
                if with_bias:
                    nc.tensor.matmul(
                        pt,
                        bqkv_sb[0:1, ft * P:(ft + 1) * P],
                        ones512_sb[0:1, :],
                        start=False, stop=True,
                    )
                # b0 casts ride the idle Scalar engine (pre-attention phase)
                dst = qkT[:, ft, tcc * 512:(tcc + 1) * 512] if ft < 2 \
                    else vT[:, tcc * 512:(tcc + 1) * 512]
                if b == 0:
                    nc.scalar.copy(dst, pt)
                else:
                    nc.vector.tensor_copy(dst, pt)

            def vt_unit(b, st):
                """PE-transpose one [128,128] v^T tile into V[s,d] layout."""
                pt = ps_c.tile([P, P], BF16, tag="c", name=f"vtps_{b}_{st}")
                nc.tensor.transpose(
                    pt, vT[:, b * T + st * P: b * T + (st + 1) * P], ident_sb)
                for h in range(HPC):
                    vidx = (b * HPC + h) * NST + st
                    nc.vector.tensor_copy(
                        V[:, vidx, 0:64], pt[:, h * 64:(h + 1) * 64])

            proj_cast_engines = [nc.vector, nc.scalar, nc.vector, nc.scalar]
            proj_count = [0]

            def proj_unit(b, of, ck):
                """One [128c, 512t] c_proj partial: MM -> cast -> DMA."""
                po = ps_c.tile([P, 512], F32, tag="c", name=f"ops_{b}_{of}_{ck}")
                nc.tensor.matmul(
                    po,
                    wp_sb[:, of * P:(of + 1) * P],
                    yT[:, b, ck * 512:(ck + 1) * 512],
                    start=True, stop=True,
                )
                ot = pwork.tile([P, 512], BF16, tag="ot", bufs=6,
                                name=f"ot_{b}_{of}_{ck}")
                eng = proj_cast_engines[proj_count[0] % 4]
                proj_count[0] += 1
                if eng is nc.scalar:
                    eng.copy(ot, po)
                else:
                    eng.tensor_copy(ot, po)
                nc.sync.dma_start(
                    outT[b, of * P:(of + 1) * P, ck * 512:(ck + 1) * 512],
                    ot,
                )

            # ---- fill-work queue (PE units interleaved into attention) ----
            fillers = []

            def pop_fillers(n):
                for _ in range(min(n, len(fillers))):
                    f, *a = fillers.pop(0)
                    f(*a)

            proj_ready = []

            def sprinkle_proj(n):
                for _ in range(min(n, len(proj_ready))):
                    proj_unit(*proj_ready.pop(0))

            # ---- attention window ----
            def attention(b, w, pending_norm):
                """Emit window w of batch b. pending_norm: list of staged
                closures from the previous window's normalization."""
                q0 = w * QW
                n_st = (q0 + QW) // P
                ys = {}
                for h in range(HPC):
                    ys[h] = ps_y.tile([P, QW], F32, tag="y",
                                      name=f"yps_{b}_{w}_{h}")
                av_fifo = []
                for st in range(n_st):
                    s0 = st * P
                    qa = max(q0, s0)
                    wa = q0 + QW - qa          # active width
                    sp = ps_s.tile([P, 2 * QW], F32, tag="s",
                                   name=f"sps_{b}_{w}_{st}")
                    for h in range(HPC):
                        nc.tensor.matmul(
                            sp[:, h * QW: h * QW + wa],
                            qkT[h * 64:(h + 1) * 64, 1,
                                b * T + s0: b * T + s0 + P],
                            qkT[h * 64:(h + 1) * 64, 0,
                                b * T + qa: b * T + qa + wa],
                            start=True, stop=True,
                        )
                    es = pwork.tile([P, 2, QW], BF16, tag="es", bufs=8,
                                    name=f"es_{b}_{w}_{st}")
                    if wa == QW:
                        nc.scalar.activation(
                            es.rearrange("p t w -> p (t w)"),
                            sp, EXP, bias=expb)
                    else:
                        nc.scalar.activation(
                            es[:, :, 0:wa],
                            sp.rearrange("p (t w) -> p t w", t=2)[:, :, 0:wa],
                            EXP, bias=expb)
                    if s0 >= q0:               # diagonal tile: causal mask
                        nc.gpsimd.tensor_mul(es[:, 0, 0:P], es[:, 0, 0:P],
                                             tri_sb)
                        nc.gpsimd.tensor_mul(es[:, 1, 0:P], es[:, 1, 0:P],
                                             tri_sb)

                    def av(st=st, es=es, off=qa - q0, wa=wa):
                        for h in range(HPC):
                            nc.tensor.matmul(
                                ys[h][:, off:QW],
                                V[:, (b * HPC + h) * NST + st, :],
                                es[:, h, 0:wa],
                                start=(st == 0),
                                stop=(st == n_st - 1),
                            )
                    av_fifo.append(av)
                    if len(av_fifo) > 3:
                        av_fifo.pop(0)()
                    if pending_norm and st >= 1:
                        pending_norm.pop(0)()
                    n_fill = (3 if (b == 0 and w == 0) else
                              2 if (b == 0 and w == 1) else 1)
                    if fillers:
                        pop_fillers(n_fill)
                        sprinkle_proj(1)
                    elif proj_ready:
                        sprinkle_proj(2)
                    else:
                        dummy_heartbeat()
                for f in av_fifo:
                    f()
                return norm_start(b, w, ys)

            def norm_start(b, w, ys):
                """Emit Z extraction now; return staged closures for the
                rest of the normalization chain."""
                q0 = w * QW
                zbs = []
                for h in range(HPC):
                    zb = pwork.tile([64, QW], F32, tag=f"zb_{h}", bufs=2,
                                    name=f"zb_{b}_{w}_{h}")
                    nc.vector.tensor_copy(zb, ys[h][64:128, :])
                    zbs.append(zb)

                def step1():
                    for h in range(HPC):
                        rinv = pwork.tile([64, QW], F32, tag=f"ri_{h}", bufs=2,
                                          name=f"ri_{b}_{w}_{h}")
                        nc.vector.reciprocal_approx_fast(rinv, zbs[h])
                        state[h] = rinv

                def step2():
                    for h in range(HPC):
                        nc.vector.tensor_mul(
                            yT[h * 64:(h + 1) * 64, b, q0:q0 + QW],
                            ys[h][0:64, :],
                            state[h],
                        )
                    proj_ready.extend((b, of, w) for of in range(NCT))

                state = {}
                return [step1, step2]

            # ---- emission schedule ----
            dummy_mms(12)
            # b0 chunk 0 QKV inline (gates window 0); everything else becomes
            # filler work popped inside the attention windows
            for ft in (1, 0, 2):
                qkv_unit(0, ft, 0)
                dummy_mms(5)
            for st4 in range(4):
                vt_unit(0, st4)
            for b in range(B):
                for ck in range(4):
                    if b == 0 and ck == 0:
                        continue
                    for ft in (1, 0, 2):
                        fillers.append((qkv_unit, b, ft, ck))
                    for st4 in range(4):
                        fillers.append((vt_unit, b, 4 * ck + st4))

            pending = []
            for b in range(B):
                if b == 1:
                    pop_fillers(len(fillers))   # flush any leftover b1 prep
                for w in range(NW):
                    pending = attention(b, w, pending)
            for f in pending:
                f()
            sprinkle_proj(len(proj_ready))
    nc.compile()
    return nc


_CACHE = {}


def _get_nc(with_bias: bool) -> bacc.Bacc:
    if with_bias not in _CACHE:
        _CACHE[with_bias] = build_nc(with_bias)
    return _CACHE[with_bias]


def _prep_inputs(x, w_attn, b_attn, w_proj):
    """Host-side shard + layout prep. Returns per-core in_maps."""
    xf = np.ascontiguousarray(
        np.asarray(x, dtype=np.float32).reshape(TOK, C).T
    ).astype(NPBF16)                                   # x^T [C, TOK]
    w = np.asarray(w_attn, dtype=np.float32)
    ba = np.asarray(b_attn, dtype=np.float32)
    wpj = np.asarray(w_proj, dtype=np.float32)
    scale = 1.0 / math.sqrt(D)
    with_bias = bool(np.any(ba))

    tri_np = np.triu(np.ones((P, P), dtype=np.float32)).astype(NPBF16)
    id_np = np.eye(P, dtype=np.float32).astype(NPBF16)
    ones512_np = np.ones((1, 512), dtype=np.float32).astype(NPBF16)

    in_maps = []
    for c in range(NCORES):
        lo, hi = c * HPC * D, (c + 1) * HPC * D        # 128-wide head slice
        wq = w[:, lo:hi] * scale
        wk = w[:, C + lo:C + hi]
        wv = w[:, 2 * C + lo:2 * C + hi]
        wqkv_c = np.concatenate([wq, wk, wv], axis=1).astype(NPBF16)
        wp_c = np.ascontiguousarray(wpj[lo:hi, :]).astype(NPBF16)
        m = {
            "xt": xf,
            "wqkv": wqkv_c,
            "wp": wp_c,
            "tri": tri_np,
            "ident": id_np,
        }
        if with_bias:
            bq = ba[lo:hi] * scale
            bk = ba[C + lo:C + hi]
            bv = ba[2 * C + lo:2 * C + hi]
            m["bqkv"] = np.concatenate([bq, bk, bv])[None, :].astype(NPBF16)
            m["ones512"] = ones512_np
        in_maps.append(m)
    return in_maps, with_bias


def _combine(results, b_proj):
    acc = np.zeros((B, C, T), dtype=np.float32)
    for r in results:
        acc += np.asarray(r["outT"], dtype=np.float32)
    out = np.transpose(acc, (0, 2, 1))                 # [B, T, C]
    out = out + np.asarray(b_proj, dtype=np.float32)[None, None, :]
    return np.ascontiguousarray(out.astype(np.float32))


def run(x, w_attn, b_attn, w_proj, b_proj, trace=False, trace_cores=None):
    in_maps, with_bias = _prep_inputs(x, w_attn, b_attn, w_proj)
    nc = _get_nc(with_bias)
    res = run_bass_kernel_spmd(
        nc, in_maps, core_ids=list(range(NCORES)),
        trace=trace, trace_cores=trace_cores,
    )
    return _combine(res.results, b_proj), res


def kernel(x, w_attn, b_attn, w_proj, b_proj):
    out, _ = run(x, w_attn, b_attn, w_proj, b_proj, trace=False)
    return out


# revision 3
# speedup vs baseline: 1.0041x; 1.0041x over previous
"""Causal self-attention Trainium2 kernel (8-core head-parallel), v2.

Sharding identical to v1 (2 heads x 2 batches per core, QKV column-parallel,
c_proj row-parallel with host-side reduce). Differences vs v1:
  - QK^T for the two heads runs CONCURRENTLY on the PE as two 64-row tiles
    (tile_position (0,0)/(64,0)) writing one [128, 1024] S_pair psum tile
    (h0 cols 0:512, h1 cols 512:1024).
  - Attention q-windows are 512 wide; one exp instruction covers both heads.
  - Softmax normalization: 1/Z on DVE (reciprocal_approx_fast) + a single
    K=2 block-ones broadcast matmul per window; no LN/EXP scalar chain.
  - c_proj partials are DMA'd directly from PSUM to DRAM in fp32.
  - xt DMA is chunk-major (b0 first) so attention starts early.
"""

import math

import numpy as np
import ml_dtypes

import concourse.bass as bass
from concourse import bacc
import concourse.mybir as mybir
from concourse.tile import TileContext
from concourse.bass_utils import run_bass_kernel_spmd

BF16 = mybir.dt.bfloat16
F32 = mybir.dt.float32
NPBF16 = ml_dtypes.bfloat16

P = 128
B, T, C = 2, 2048, 1024
H, D = 16, 64
NCORES = 8
HPC = H // NCORES          # heads per core
TOK = B * T                # 4096 flattened tokens (b-major)
NCT = C // P               # 8 contraction tiles for the projections
QW = 512                   # q window width
NW = T // QW               # 4 windows per batch elem
NST = T // P               # 16 s-tiles per batch elem
EXP_BIAS = -4.0            # exp(s - 4): cancels in normalization, guards tail


def _patch_act_tables():
    """Force every exp/ln activation onto the single table set that contains
    both, so the kernel never pays mid-stream ACT_TABLE_LOAD switches."""
    import concourse.bacc as bacc_mod
    if getattr(bacc_mod, "_act_tables_patched", False):
        return
    orig = bacc_mod.get_activation_tables
    EXP = mybir.ActivationFunctionType.Exp
    LN = mybir.ActivationFunctionType.Ln

    def patched(arch):
        t = orig(arch)
        if any(EXP in f and LN in f for f in t.values()):
            for name, fns in t.items():
                if "natural_log_exp" not in name and (EXP in fns or LN in fns):
                    t[name] = fns - {EXP, LN}
        return t

    bacc_mod.get_activation_tables = patched
    bacc_mod._act_tables_patched = True


def build_nc(with_bias: bool) -> bacc.Bacc:
    _patch_act_tables()
    nc = bacc.Bacc(None, target_bir_lowering=False)

    xt = nc.dram_tensor("xt", [C, TOK], BF16, kind="ExternalInput")
    wqkv = nc.dram_tensor("wqkv", [C, 3 * P], BF16, kind="ExternalInput")
    wp = nc.dram_tensor("wp", [P, C], BF16, kind="ExternalInput")
    tri = nc.dram_tensor("tri", [P, P], BF16, kind="ExternalInput")
    ident = nc.dram_tensor("ident", [P, P], BF16, kind="ExternalInput")

    if with_bias:
        bqkv = nc.dram_tensor("bqkv", [1, 3 * P], BF16, kind="ExternalInput")
        ones512 = nc.dram_tensor("ones512", [1, 512], BF16, kind="ExternalInput")
    outT = nc.dram_tensor("outT", [B, C, T], BF16, kind="ExternalOutput")

    EXP = mybir.ActivationFunctionType.Exp

    with TileContext(nc) as tc:
        with (
            tc.tile_pool(name="consts", bufs=1) as consts,
            tc.tile_pool(name="px", bufs=1) as px,
            tc.tile_pool(name="pqkv", bufs=1) as pqkv,
            tc.tile_pool(name="py", bufs=1) as py,
            tc.tile_pool(name="pwork", bufs=2) as pwork,
            tc.tile_pool(name="ps_s", bufs=2, space="PSUM") as ps_s,
            tc.tile_pool(name="ps_y", bufs=2, space="PSUM") as ps_y,
            tc.tile_pool(name="ps_c", bufs=2, space="PSUM") as ps_c,
        ):
            # ---- input loads: wqkv first (the warm-up dummies read it),
            # then xt chunk 0 (gates the first QKV), consts, remaining xt ----
            xt_sb = px.tile([P, NCT, TOK], BF16)

            def xt_dma(b, ck):
                lo = b * T + ck * 512
                for ct in range(NCT):
                    nc.sync.dma_start(
                        xt_sb[:, ct, lo:lo + 512],
                        xt[ct * P:(ct + 1) * P, lo:lo + 512])

            wqkv_sb = consts.tile([P, NCT, 3 * P], BF16)
            for ct in range(NCT):
                nc.sync.dma_start(wqkv_sb[:, ct, :], wqkv[ct * P:(ct + 1) * P, :])
            xt_dma(0, 0)
            wp_sb = consts.tile([P, C], BF16)
            nc.sync.dma_start(wp_sb, wp[:, :])
            tri_sb = consts.tile([P, P], BF16)
            nc.sync.dma_start(tri_sb, tri[:, :])
            ident_sb = consts.tile([P, P], BF16)
            nc.sync.dma_start(ident_sb, ident[:, :])
            expb = consts.tile([P, 1], F32)
            nc.vector.memset(expb, EXP_BIAS)
            if with_bias:
                bqkv_sb = consts.tile([1, 3 * P], BF16)
                nc.sync.dma_start(bqkv_sb, bqkv[:, :])
                ones512_sb = consts.tile([1, 512], BF16)
                nc.sync.dma_start(ones512_sb, ones512[:, :])
            for ck in range(1, 4):
                xt_dma(0, ck)
            for ck in range(4):
                xt_dma(1, ck)

            # qk slabs: ft=0 -> q^T (rows: h0 0:64, h1 64:128), ft=1 -> k^T
            qkT = pqkv.tile([P, 2, TOK], BF16)
            vT = pqkv.tile([P, TOK], BF16)
            # V tiles [s, d] with 64 ones columns (64:128) so the AV matmul
            # replicates the softmax denominator Z across partitions 64:127;
            # index = (b*HPC+h)*NST + st
            V = pqkv.tile([P, B * HPC * NST, P], BF16)
            nc.vector.memset(V, 1.0)
            yT = py.tile([P, B, T], BF16)

            # ---- PE unit definitions ----
            def dummy_mms(n):
                for _ in range(n):
                    scratch = ps_c.tile([P, 512], F32, tag="c",
                                        name="warm")
                    nc.tensor.matmul(
                        scratch[:, 0:384],
                        wqkv_sb[:, 0, 0:P],
                        wqkv_sb[:, 0, :],
                        start=True, stop=True,
                    )

            def dummy_heartbeat():
                # keep the PE HAM counter busy without touching PSUM
                nc.tensor.ldweights(tri_sb[:, :])

            def qkv_unit(b, ft, ck):
                """One [128, 512] slab of q^T/k^T/v^T for batch b, chunk ck."""
                tcc = b * 4 + ck
                pt = ps_c.tile([P, 512], F32, tag="c", name=f"qkvps_{b}_{ft}_{ck}")
                for ct in range(NCT):
                    nc.tensor.matmul(
                        pt,
                        wqkv_sb[:, ct, ft * P:(ft + 1) * P],
                        xt_sb[:, ct, tcc * 512:(tcc + 1) * 512],
                        start=(ct == 0),
                        stop=(ct == NCT - 1 and not with_bias),
                    )
                if with_bias:
                    nc.tensor.matmul(
                        pt,
                        bqkv_sb[0:1, ft * P:(ft + 1) * P],
                        ones512_sb[0:1, :],
                        start=False, stop=True,
                    )
                # b0 casts ride the idle Scalar engine (pre-attention phase)
                dst = qkT[:, ft, tcc * 512:(tcc + 1) * 512] if ft < 2 \
                    else vT[:, tcc * 512:(tcc + 1) * 512]
                if b == 0:
                    nc.scalar.copy(dst, pt)
                else:
                    nc.vector.tensor_copy(dst, pt)

            def vt_unit(b, st):
                """PE-transpose one [128,128] v^T tile into V[s,d] layout."""
                pt = ps_c.tile([P, P], BF16, tag="c", name=f"vtps_{b}_{st}")
                nc.tensor.transpose(
                    pt, vT[:, b * T + st * P: b * T + (st + 1) * P], ident_sb)
                for h in range(HPC):
                    vidx = (b * HPC + h) * NST + st
                    nc.vector.tensor_copy(
                        V[:, vidx, 0:64], pt[:, h * 64:(h + 1) * 64])

            proj_cast_engines = [nc.vector, nc.scalar, nc.vector, nc.scalar]
            proj_count = [0]

            def proj_unit(b, of, ck):
                """One [128c, 512t] c_proj partial: MM -> cast -> DMA."""
                po = ps_c.tile([P, 512], F32, tag="c", name=f"ops_{b}_{of}_{ck}")
                nc.tensor.matmul(
                    po,
                    wp_sb[:, of * P:(of + 1) * P],
                    yT[:, b, ck * 512:(ck + 1) * 512],
                    start=True, stop=True,
                )
                ot = pwork.tile([P, 512], BF16, tag="ot", bufs=6,
                                name=f"ot_{b}_{of}_{ck}")
                eng = proj_cast_engines[proj_count[0] % 4]
                proj_count[0] += 1
                if eng is nc.scalar:
                    eng.copy(ot, po)
                else:
                    eng.tensor_copy(ot, po)
                nc.sync.dma_start(
                    outT[b, of * P:(of + 1) * P, ck * 512:(ck + 1) * 512],
                    ot,
                )

            # ---- fill-work queue (PE units interleaved into attention) ----
            fillers = []

            def pop_fillers(n):
                for _ in range(min(n, len(fillers))):
                    f, *a = fillers.pop(0)
                    f(*a)

            proj_ready = []

            def sprinkle_proj(n):
                for _ in range(min(n, len(proj_ready))):
                    proj_unit(*proj_ready.pop(0))

            # ---- attention window ----
            def attention(b, w, pending_norm):
                """Emit window w of batch b. pending_norm: list of staged
                closures from the previous window's normalization."""
                q0 = w * QW
                n_st = (q0 + QW) // P
                ys = {}
                for h in range(HPC):
                    ys[h] = ps_y.tile([P, QW], F32, tag="y",
                                      name=f"yps_{b}_{w}_{h}")
                av_fifo = []
                for st in range(n_st):
                    s0 = st * P
                    qa = max(q0, s0)
                    wa = q0 + QW - qa          # active width
                    sp = ps_s.tile([P, 2 * QW], F32, tag="s",
                                   name=f"sps_{b}_{w}_{st}")
                    for h in range(HPC):
                        nc.tensor.matmul(
                            sp[:, h * QW: h * QW + wa],
                            qkT[h * 64:(h + 1) * 64, 1,
                                b * T + s0: b * T + s0 + P],
                            qkT[h * 64:(h + 1) * 64, 0,
                                b * T + qa: b * T + qa + wa],
                            start=True, stop=True,
                        )
                    es = pwork.tile([P, 2, QW], BF16, tag="es", bufs=8,
                                    name=f"es_{b}_{w}_{st}")
                    if wa == QW:
                        nc.scalar.activation(
                            es.rearrange("p t w -> p (t w)"),
                            sp, EXP, bias=expb)
                    else:
                        nc.scalar.activation(
                            es[:, :, 0:wa],
                            sp.rearrange("p (t w) -> p t w", t=2)[:, :, 0:wa],
                            EXP, bias=expb)
                    if s0 >= q0:               # diagonal tile: causal mask
                        nc.gpsimd.tensor_mul(es[:, 0, 0:P], es[:, 0, 0:P],
                                             tri_sb)
                        nc.gpsimd.tensor_mul(es[:, 1, 0:P], es[:, 1, 0:P],
                                             tri_sb)

                    def av(st=st, es=es, off=qa - q0, wa=wa):
                        for h in range(HPC):
                            nc.tensor.matmul(
                                ys[h][:, off:QW],
                                V[:, (b * HPC + h) * NST + st, :],
                                es[:, h, 0:wa],
                                start=(st == 0),
                                stop=(st == n_st - 1),
                            )
                    av_fifo.append(av)
                    if len(av_fifo) > 3:
                        av_fifo.pop(0)()
                    if pending_norm and st >= 1:
                        pending_norm.pop(0)()
                    n_fill = (3 if (b == 0 and w == 0) else
                              2 if (b == 0 and w == 1) else 1)
                    if fillers:
                        pop_fillers(n_fill)
                        sprinkle_proj(1)
                    elif proj_ready:
                        sprinkle_proj(2)
                    else:
                        dummy_heartbeat()
                for f in av_fifo:
                    f()
                return norm_start(b, w, ys)

            def norm_start(b, w, ys):
                """Emit Z extraction now; return staged closures for the
                rest of the normalization chain."""
                q0 = w * QW
                zbs = []
                for h in range(HPC):
                    zb = pwork.tile([64, QW], F32, tag=f"zb_{h}", bufs=2,
                                    name=f"zb_{b}_{w}_{h}")
                    nc.vector.tensor_copy(zb, ys[h][64:128, :])
                    zbs.append(zb)

                def step1():
                    for h in range(HPC):
                        rinv = pwork.tile([64, QW], F32, tag=f"ri_{h}", bufs=2,
                                          name=f"ri_{b}_{w}_{h}")
                        nc.vector.reciprocal_approx_fast(rinv, zbs[h])
                        state[h] = rinv

                def step2():
                    for h in range(HPC):
                        nc.vector.tensor_mul(
                            yT[h * 64:(h + 1) * 64, b, q0:q0 + QW],
                            ys[h][0:64, :],
                            state[h],
                        )
                    proj_ready.extend((b, of, w) for of in range(NCT))

                state = {}
                return [step1, step2]

            # ---- emission schedule ----
            dummy_mms(16)
            # b0 chunk 0 QKV inline (gates window 0); everything else becomes
            # filler work popped inside the attention windows
            for ft in (1, 0, 2):
                qkv_unit(0, ft, 0)
                dummy_mms(6)
            for st4 in range(4):
                vt_unit(0, st4)
            for b in range(B):
                for ck in range(4):
                    if b == 0 and ck == 0:
                        continue
                    for ft in (1, 0, 2):
                        fillers.append((qkv_unit, b, ft, ck))
                    for st4 in range(4):
                        fillers.append((vt_unit, b, 4 * ck + st4))

            pending = []
            for b in range(B):
                if b == 1:
                    pop_fillers(len(fillers))   # flush any leftover b1 prep
                for w in range(NW):
                    pending = attention(b, w, pending)
            for f in pending:
                f()
            sprinkle_proj(len(proj_ready))
    nc.compile()
    return nc


_CACHE = {}


def _get_nc(with_bias: bool) -> bacc.Bacc:
    if with_bias not in _CACHE:
        _CACHE[with_bias] = build_nc(with_bias)
    return _CACHE[with_bias]


def _prep_inputs(x, w_attn, b_attn, w_proj):
    """Host-side shard + layout prep. Returns per-core in_maps."""
    xf = np.ascontiguousarray(
        np.asarray(x, dtype=np.float32).reshape(TOK, C).T
    ).astype(NPBF16)                                   # x^T [C, TOK]
    w = np.asarray(w_attn, dtype=np.float32)
    ba = np.asarray(b_attn, dtype=np.float32)
    wpj = np.asarray(w_proj, dtype=np.float32)
    scale = 1.0 / math.sqrt(D)
    with_bias = bool(np.any(ba))

    tri_np = np.triu(np.ones((P, P), dtype=np.float32)).astype(NPBF16)
    id_np = np.eye(P, dtype=np.float32).astype(NPBF16)
    ones512_np = np.ones((1, 512), dtype=np.float32).astype(NPBF16)

    in_maps = []
    for c in range(NCORES):
        lo, hi = c * HPC * D, (c + 1) * HPC * D        # 128-wide head slice
        wq = w[:, lo:hi] * scale
        wk = w[:, C + lo:C + hi]
        wv = w[:, 2 * C + lo:2 * C + hi]
        wqkv_c = np.concatenate([wq, wk, wv], axis=1).astype(NPBF16)
        wp_c = np.ascontiguousarray(wpj[lo:hi, :]).astype(NPBF16)
        m = {
            "xt": xf,
            "wqkv": wqkv_c,
            "wp": wp_c,
            "tri": tri_np,
            "ident": id_np,
        }
        if with_bias:
            bq = ba[lo:hi] * scale
            bk = ba[C + lo:C + hi]
            bv = ba[2 * C + lo:2 * C + hi]
            m["bqkv"] = np.concatenate([bq, bk, bv])[None, :].astype(NPBF16)
            m["ones512"] = ones512_np
        in_maps.append(m)
    return in_maps, with_bias


def _combine(results, b_proj):
    acc = np.zeros((B, C, T), dtype=np.float32)
    for r in results:
        acc += np.asarray(r["outT"], dtype=np.float32)
    out = np.transpose(acc, (0, 2, 1))                 # [B, T, C]
    out = out + np.asarray(b_proj, dtype=np.float32)[None, None, :]
    return np.ascontiguousarray(out.astype(np.float32))


def run(x, w_attn, b_attn, w_proj, b_proj, trace=False, trace_cores=None):
    in_maps, with_bias = _prep_inputs(x, w_attn, b_attn, w_proj)
    nc = _get_nc(with_bias)
    res = run_bass_kernel_spmd(
        nc, in_maps, core_ids=list(range(NCORES)),
        trace=trace, trace_cores=trace_cores,
    )
    return _combine(res.results, b_proj), res


def kernel(x, w_attn, b_attn, w_proj, b_proj):
    out, _ = run(x, w_attn, b_attn, w_proj, b_proj, trace=False)
    return out
